# revision 5
# baseline (speedup 1.0000x reference)
"""Tensor-parallel Trainium2 Bass kernel for the pre-LN decoder block.

Sharding (8 cores):
  - tokens: core c owns tokens [512c : 512(c+1)) of the flattened [B*T]=4096
    stream (batch b = c//4, chunk ch = c%4) for LN / residual / output.
  - heads: core c owns heads {2c, 2c+1} for attention over ALL tokens.
  - MLP hidden: core c owns hidden cols [512c : 512(c+1)) for ALL tokens.

Flow: LN1(local) -> AllGather lnx -> QKV(2 heads, all tokens) -> causal
attention (identical work on every core) -> partial W_o -> ReduceScatter
(x2 partial sums, split in two halves by feature tile) -> residual + LN2
(local) -> AllGather ln2 -> MLP hidden slice (all tokens) -> partial proj
-> ReduceScatter -> residual -> output (local 512 tokens).

Everything is bf16 on the wire and in matmuls (fp32 PSUM accumulate);
softmax denominator via an extra ones-column in V; causal masking via an
additive -1e9 mask on the 4 diagonal key-tiles of each query chunk.
"""

import sys

if "/opt/trn_rl_repo" not in sys.path:
    sys.path.insert(0, "/opt/trn_rl_repo")

import ml_dtypes
import numpy as np

N_EMBD = 1024
N_HEAD = 16
HEAD_DIM = 64
B, T = 2, 2048
NC = 8
ET = N_EMBD // 128  # 8 feature tiles
TOK = 512  # tokens per core
NTOK = B * T  # 4096
NKT = NTOK // 128  # 32 key tiles over both batches
EPS = 1e-5


def build(stage=3, dbg=False, reps=1):
    import concourse.mybir as mybir
    import concourse.tile as tile
    from concourse import bacc

    f32 = mybir.dt.float32
    bf16 = mybir.dt.bfloat16
    f8 = mybir.dt.float8e4

    nc = bacc.Bacc("TRN2", target_bir_lowering=False, debug=False, num_devices=NC)

    g = {}
    g["xT"] = nc.declare_dram_parameter("xT", [ET, 128, TOK], bf16, isOutput=False)
    g["wq"] = nc.declare_dram_parameter("wq", [128, ET, 128], bf16, isOutput=False)
    g["wk"] = nc.declare_dram_parameter("wk", [128, ET, 128], bf16, isOutput=False)
    g["wv"] = nc.declare_dram_parameter("wv", [128, ET, 128], bf16, isOutput=False)
    g["wo"] = nc.declare_dram_parameter("wo", [128, ET, 128], bf16, isOutput=False)
    g["wfc"] = nc.declare_dram_parameter("wfc", [128, ET, 4, 128], bf16, isOutput=False)
    g["wpr"] = nc.declare_dram_parameter("wpr", [128, 4, ET, 128], bf16, isOutput=False)
    g["bq"] = nc.declare_dram_parameter("bq", [128, 1], f32, isOutput=False)
    g["bk"] = nc.declare_dram_parameter("bk", [128, 1], f32, isOutput=False)
    g["bo2"] = nc.declare_dram_parameter("bo2", [128, ET], f32, isOutput=False)
    g["bfc"] = nc.declare_dram_parameter("bfc", [128, 4], f32, isOutput=False)
    g["bpr"] = nc.declare_dram_parameter("bpr", [128, ET], f32, isOutput=False)
    for nm in ("g1", "b1", "g2", "b2"):
        g[nm] = nc.declare_dram_parameter(nm, [128, ET], f32, isOutput=False)
    g["dmask"] = nc.declare_dram_parameter("dmask", [128, 896], f32, isOutput=False)

    g["yT"] = nc.declare_dram_parameter("yT", [ET, 128, TOK], f32, isOutput=True)
    dbg_out = {}
    if dbg:
        dbg_out["ln"] = nc.declare_dram_parameter("d_ln", [ET, 128, TOK], mybir.dt.float8e4, isOutput=True)
        dbg_out["qT"] = nc.declare_dram_parameter("d_qT", [128, NTOK], bf16, isOutput=True)
        dbg_out["kT"] = nc.declare_dram_parameter("d_kT", [128, NTOK], bf16, isOutput=True)
        dbg_out["v"] = nc.declare_dram_parameter("d_v", [NKT, 128, 130], bf16, isOutput=True)
        if stage >= 2:
            dbg_out["attnT"] = nc.declare_dram_parameter("d_attnT", [128, NTOK], bf16, isOutput=True)
            dbg_out["x2T"] = nc.declare_dram_parameter("d_x2T", [ET, 128, TOK], f32, isOutput=True)
            dbg_out["ln2"] = nc.declare_dram_parameter("d_ln2", [ET, 128, TOK], mybir.dt.float8e4, isOutput=True)

    # --- DRAM bounce buffers for collectives (outputs pair-HBM Shared) ---
    cc = {}
    for h in ("A", "B"):
        cc[f"lnx_in{h}"] = nc.dram_tensor(f"lnx_in{h}", [ET, 128, TOK // 2], f8)
        cc[f"lnx_ag{h}"] = nc.dram_tensor(f"lnx_ag{h}", [NC, ET, 128, TOK // 2], f8, addr_space="Shared")
        cc[f"ln2_in{h}"] = nc.dram_tensor(f"ln2_in{h}", [ET, 128, TOK // 2], f8)
        cc[f"ln2_ag{h}"] = nc.dram_tensor(f"ln2_ag{h}", [NC, ET, 128, TOK // 2], f8, addr_space="Shared")
    cc["x2_in"] = nc.dram_tensor("x2i", [NC, ET, 128, TOK], bf16)
    cc["x2_out"] = nc.dram_tensor("x2o", [ET, 128, TOK], bf16)
    cc["y_in"] = nc.dram_tensor("yi", [NC, ET, 128, TOK], bf16)
    cc["y_out"] = nc.dram_tensor("yo", [ET, 128, TOK], bf16)
    g["cc"] = cc

    with tile.TileContext(nc) as tc:
        with tc.tile_pool(name="const", bufs=1) as cpool:

            # --- constants / weights resident in SBUF, loaded once ---
            ones_tmp = cpool.tile([128, 128], bf16, tag="ones_tmp", name="ones_tmp")
            nc.vector.memset(ones_tmp[:, :], 1.0)
            g["ones_p"] = cpool.tile([128, 1], bf16, tag="ones_p", name="ones_p")
            g["ones_f"] = cpool.tile([1, 128], bf16, tag="ones_f", name="ones_f")
            nc.vector.tensor_copy(g["ones_p"][:, :], ones_tmp[:, 0:1])
            nc.vector.tensor_copy(g["ones_f"][:, :], ones_tmp[0:1, :])
            g["eps_sb"] = cpool.tile([128, 1], f32, tag="eps_sb", name="eps_sb")
            nc.vector.memset(g["eps_sb"][:, :], EPS)

            for nm, shp, dt_ in (
                ("wq_sb", [128, ET, 128], bf16), ("wk_sb", [128, ET, 128], bf16),
                ("wv_sb", [128, ET, 128], bf16), ("wo_sb", [128, ET, 128], bf16),
                ("wfc_sb", [128, ET, 4, 128], bf16), ("wpr_sb", [128, 4, ET, 128], bf16),
                ("bq_sb", [128, 1], f32), ("bk_sb", [128, 1], f32),
                ("bo2_sb", [128, ET], f32), ("bfc_sb", [128, 4], f32),
                ("bpr_sb", [128, ET], f32),
                ("g1_sb", [128, ET], f32), ("b1_sb", [128, ET], f32),
                ("g2_sb", [128, ET], f32), ("b2_sb", [128, ET], f32),
                ("mask_sb", [128, 896], f32),
            ):
                g[nm] = cpool.tile(shp, dt_, tag=nm, name=nm)
            for s_, t_ in (("wq_sb", "wq"), ("wk_sb", "wk"), ("wv_sb", "wv"),
                           ("wo_sb", "wo"), ("wfc_sb", "wfc"), ("wpr_sb", "wpr"),
                           ("bq_sb", "bq"), ("bk_sb", "bk"), ("bo2_sb", "bo2"),
                           ("bfc_sb", "bfc"), ("bpr_sb", "bpr"),
                           ("g1_sb", "g1"), ("b1_sb", "b1"),
                           ("g2_sb", "g2"), ("b2_sb", "b2")):
                src = g[t_]
                src_ap = src[tuple(slice(None) for _ in src.shape)]
                dst = g[s_]
                nc.sync.dma_start(out=dst[tuple(slice(None) for _ in dst.shape)],
                                  in_=src_ap)
            nc.sync.dma_start(out=g["mask_sb"][:, :], in_=g["dmask"][:, :])

            with nc.allow_low_precision(reason="bf16 matmuls/collectives by design"):
                for _rep in range(reps):
                    _body(nc, tc, g, stage, dbg, dbg_out)

    nc.compile()
    return nc, dbg_out


def _body(nc, tc, g, stage, dbg, dbg_out):
    import concourse.mybir as mybir

    f32 = mybir.dt.float32
    bf16 = mybir.dt.bfloat16
    f8 = mybir.dt.float8e4
    AF = mybir.ActivationFunctionType
    OP = mybir.AluOpType

    cc = g["cc"]
    ones_p, ones_f = g["ones_p"], g["ones_f"]

    with tc.tile_pool(name="res", bufs=1) as rpool:
        qT = rpool.tile([128, NTOK], bf16, tag="qT", name="qT")
        kT = rpool.tile([128, NTOK], bf16, tag="kT", name="kT")
        attnT = rpool.tile([128, NTOK], bf16, tag="attnT", name="attnT")
        v_sb = [rpool.tile([128, 130], bf16, tag=f"v{i}", name=f"v{i}") for i in range(NKT)]
        x_sb = [rpool.tile([128, TOK], bf16, tag=f"x{i}", name=f"x{i}") for i in range(ET)]
        x2T = [rpool.tile([128, TOK], f32, tag=f"x2T{i}", name=f"x2T{i}") for i in range(ET)]
        h_sb = [rpool.tile([128, NTOK], bf16, tag=f"h{i}", name=f"h{i}") for i in range(4)]
        ln1_t = [rpool.tile([128, TOK], f8, tag=f"l1{i}", name=f"l1{i}") for i in range(ET)]
        ln2_t = [rpool.tile([128, TOK], f8, tag=f"l2{i}", name=f"l2{i}") for i in range(ET)]

        def layernorm_local(pool, pspool, src_f32, src_b16, g_sb, b_sb, out_t, pfx):
            """LN over features (partitions).  src_b16 feeds the sum matmuls;
            src_f32 (may be the same tiles) feeds the subtract path."""
            ps_s = pspool.tile([1, TOK], f32, tag=pfx + "s", name=pfx + "s")
            ps_q = pspool.tile([1, TOK], f32, tag=pfx + "q", name=pfx + "q")
            for et in range(ET):
                sq = pool.tile([128, TOK], bf16, tag=pfx + "sq", name=pfx + "sq")
                nc.vector.tensor_mul(sq[:, :], src_f32[et][:, :], src_f32[et][:, :])
                nc.tensor.matmul(ps_s[:, :], ones_p[:, :], src_b16[et][:, :],
                                 start=(et == 0), stop=(et == ET - 1))
                nc.tensor.matmul(ps_q[:, :], ones_p[:, :], sq[:, :],
                                 start=(et == 0), stop=(et == ET - 1))
            mu = pool.tile([1, TOK], f32, tag=pfx + "mu", name=pfx + "mu")
            nc.scalar.activation(mu[:, :], ps_s[:, :], AF.Copy, scale=1.0 / N_EMBD)
            musq = pool.tile([1, TOK], f32, tag=pfx + "musq", name=pfx + "musq")
            nc.vector.tensor_mul(musq[:, :], mu[:, :], mu[:, :])
            var = pool.tile([1, TOK], f32, tag=pfx + "var", name=pfx + "var")
            nc.scalar.activation(var[:, :], ps_q[:, :], AF.Copy, scale=1.0 / N_EMBD)
            nc.vector.tensor_sub(var[:, :], var[:, :], musq[:, :])
            sd = pool.tile([1, TOK], f32, tag=pfx + "sd", name=pfx + "sd")
            nc.scalar.activation(sd[:, :], var[:, :], AF.Sqrt, bias=g["eps_sb"][0:1, 0:1])
            rstd = pool.tile([1, TOK], f32, tag=pfx + "rstd", name=pfx + "rstd")
            nc.vector.reciprocal(rstd[:, :], sd[:, :])
            mu_h = pool.tile([1, TOK], bf16, tag=pfx + "muh", name=pfx + "muh")
            rs_h = pool.tile([1, TOK], bf16, tag=pfx + "rsh", name=pfx + "rsh")
            nc.vector.tensor_copy(mu_h[:, :], mu[:, :])
            nc.vector.tensor_copy(rs_h[:, :], rstd[:, :])
            ps_mb = pspool.tile([128, TOK], f32, tag=pfx + "mb", name=pfx + "mb")
            ps_rb = pspool.tile([128, TOK], f32, tag=pfx + "rb", name=pfx + "rb")
            nc.tensor.matmul(ps_mb[:, :], ones_f[:, :], mu_h[:, :], start=True, stop=True)
            nc.tensor.matmul(ps_rb[:, :], ones_f[:, :], rs_h[:, :], start=True, stop=True)
            mu_b = pool.tile([128, TOK], f32, tag=pfx + "mu_b", name=pfx + "mu_b")
            rs_b = pool.tile([128, TOK], f32, tag=pfx + "rs_b", name=pfx + "rs_b")
            nc.vector.tensor_copy(mu_b[:, :], ps_mb[:, :])
            nc.vector.tensor_copy(rs_b[:, :], ps_rb[:, :])
            for et in range(ET):
                t1 = pool.tile([128, TOK], f32, tag=pfx + "t1", name=pfx + "t1")
                nc.vector.tensor_sub(t1[:, :], src_f32[et][:, :], mu_b[:, :])
                nc.vector.tensor_mul(t1[:, :], t1[:, :], rs_b[:, :])
                nc.vector.tensor_scalar(out_t[et][:, :], t1[:, :],
                                        g_sb[:, et:et + 1], b_sb[:, et:et + 1],
                                        OP.mult, OP.add)

        # ---------------- Phase A: LN1 + AllGather ----------------
        with (
            tc.tile_pool(name="pa", bufs=2) as pa,
            tc.tile_pool(name="pa_ps", bufs=1, space="PSUM") as paps,
        ):
            for et in range(ET):
                nc.sync.dma_start(out=x_sb[et][:, :], in_=g["xT"][et, :, :])
            layernorm_local(pa, paps, x_sb, x_sb, g["g1_sb"], g["b1_sb"], ln1_t, "a")
            for hi, half in enumerate(("A", "B")):
                hs = slice(256 * hi, 256 * (hi + 1))
                for et in range(ET):
                    nc.sync.dma_start(out=cc[f"lnx_in{half}"][et, :, :],
                                      in_=ln1_t[et][:, hs])
                nc.gpsimd.collective_compute(
                    "AllGather", mybir.AluOpType.bypass,
                    replica_groups=[list(range(NC))],
                    ins=[cc[f"lnx_in{half}"][:, :, :]],
                    outs=[cc[f"lnx_ag{half}"][:, :, :, :]])

        # ---------------- Phase B: QKV for my 2 heads, V token-major ----------------
        with (
            tc.tile_pool(name="pb_mv", bufs=2) as mvpool,
            tc.tile_pool(name="pb_sb", bufs=3) as pb,
            tc.tile_pool(name="pb_ps", bufs=4, space="PSUM") as qps,
        ):
            for hi, half in enumerate(("A", "B")):
                for c8 in range(NC):
                    mv8 = mvpool.tile([128, ET, TOK // 2], f8, tag="mv8", name="mv8")
                    nc.sync.dma_start(out=mv8[:, :, :],
                                      in_=cc[f"lnx_ag{half}"][c8].transpose([1, 0, 2]))
                    mv = mvpool.tile([128, ET, TOK // 2], bf16, tag="mv", name="mv")
                    nc.vector.tensor_copy(mv[:, :, :], mv8[:, :, :])
                    sl = slice(TOK * c8 + 256 * hi, TOK * c8 + 256 * (hi + 1))
                    for wt, bias, dest in ((g["wq_sb"], g["bq_sb"], qT),
                                           (g["wk_sb"], g["bk_sb"], kT)):
                        ps = qps.tile([128, TOK // 2], f32, tag="qkps", name="qkps")
                        for et in range(ET):
                            nc.tensor.matmul(ps[:, :], wt[:, et, :], mv[:, et, :],
                                             start=(et == 0), stop=(et == ET - 1))
                        nc.vector.tensor_scalar(dest[:, sl], ps[:, :],
                                                bias[:, 0:1], None, OP.add)
                    for kt in range(2):
                        ps = qps.tile([128, 128], f32, tag="vps", name="vps")
                        for et in range(ET):
                            nc.tensor.matmul(ps[:, :], mv[:, et, 128 * kt:128 * (kt + 1)],
                                             g["wv_sb"][:, et, :],
                                             start=(et == 0), stop=(et == ET - 1))
                        kti = 4 * c8 + 2 * hi + kt
                        nc.vector.tensor_copy(v_sb[kti][:, 0:64], ps[:, 0:64])
                        nc.scalar.activation(v_sb[kti][:, 65:129], ps[:, 64:128], AF.Copy)
            for kti in range(NKT):
                nc.gpsimd.memset(v_sb[kti][:, 64:65], 1.0)
                nc.gpsimd.memset(v_sb[kti][:, 129:130], 1.0)
            if dbg:
                for et in range(ET):
                    nc.sync.dma_start(out=dbg_out["ln"][et, :, :], in_=ln1_t[et][:, :])
                nc.sync.dma_start(out=dbg_out["qT"][:, :], in_=qT[:, :])
                nc.sync.dma_start(out=dbg_out["kT"][:, :], in_=kT[:, :])
                for kti in range(NKT):
                    nc.sync.dma_start(out=dbg_out["v"][kti, :, :], in_=v_sb[kti][:, :])

        if stage < 2:
            for et in range(ET):
                nc.gpsimd.dma_start(out=g["yT"][et, :, :], in_=ln1_t[et][:, :])
            return

        # ---------------- Phase C: attention (2 heads x 2 batches x 4 qchunks) ----------------
        with (
            tc.tile_pool(name="pc_sb", bufs=3) as ap,
            tc.tile_pool(name="pc_exp", bufs=3) as epool,
            tc.tile_pool(name="pc_ps", bufs=2, space="PSUM") as sps,
            tc.tile_pool(name="pc_av", bufs=2, space="PSUM") as avps,
            tc.tile_pool(name="pc_bc", bufs=2, space="PSUM") as bcps,
        ):
            for b in range(2):
                for h in range(2):
                    po = 64 * h
                    for qc in range(4):
                        nkt = 4 * (qc + 1)
                        qsl = slice(2048 * b + 512 * qc, 2048 * b + 512 * (qc + 1))
                        ps_a = avps.tile([65, TOK], f32, tag="av", name="av")
                        for kt2 in range(nkt // 2):
                            ps_s = sps.tile([128, 1024], f32, tag="sc", name="sc")
                            ex = epool.tile([128, 1024], bf16, tag="ex", name="ex")
                            for j in range(2):
                                kt = 2 * kt2 + j
                                ksl = slice(2048 * b + 128 * kt, 2048 * b + 128 * (kt + 1))
                                nc.tensor.matmul(ps_s[:, 512 * j:512 * (j + 1)],
                                                 kT[po:po + 64, ksl],
                                                 qT[po:po + 64, qsl],
                                                 start=True, stop=True)
                                if kt >= 4 * qc:
                                    o = kt - 4 * qc
                                    ms = 384 - 128 * o
                                    nc.vector.tensor_add(ps_s[:, 512 * j:512 * (j + 1)],
                                                         ps_s[:, 512 * j:512 * (j + 1)],
                                                         g["mask_sb"][:, ms:ms + TOK])
                            nc.scalar.activation(ex[:, :], ps_s[:, :], AF.Exp)
                            for j in range(2):
                                kt = 2 * kt2 + j
                                kti = 16 * b + kt
                                nc.tensor.matmul(ps_a[:, :],
                                                 v_sb[kti][:, 65 * h:65 * h + 65],
                                                 ex[:, 512 * j:512 * (j + 1)],
                                                 start=(kt == 0), stop=(kt == nkt - 1),
                                                 skip_group_check=True)
                        recip = ap.tile([1, TOK], bf16, tag="rc", name="rc")
                        nc.vector.reciprocal(recip[:, :], ps_a[64:65, :])
                        ps_b = bcps.tile([64, TOK], f32, tag="bc", name="bc")
                        nc.tensor.matmul(ps_b[:, :], ones_f[0:1, 0:64], recip[:, :],
                                         start=True, stop=True)
                        rb = ap.tile([64, TOK], f32, tag="rb", name="rb")
                        nc.scalar.activation(rb[:, :], ps_b[:, :], AF.Copy)
                        nc.vector.tensor_mul(attnT[po:po + 64, qsl],
                                             ps_a[0:64, :], rb[:, :])
            if dbg:
                nc.sync.dma_start(out=dbg_out["attnT"][:, :], in_=attnT[:, :])

        # ---------------- Phase D: partial W_o + ReduceScatter + residual + LN2 ----------------
        with (
            tc.tile_pool(name="pd_sb", bufs=4) as pd,
            tc.tile_pool(name="pd_ps", bufs=4, space="PSUM") as dps,
        ):
            for et in range(ET):
                for c8 in range(NC):
                    ps = dps.tile([128, TOK], f32, tag="wops", name="wops")
                    nc.tensor.matmul(ps[:, :], g["wo_sb"][:, et, :],
                                     attnT[:, TOK * c8:TOK * (c8 + 1)],
                                     start=True, stop=True)
                    xp = pd.tile([128, TOK], bf16, tag="xp", name="xp")
                    if c8 % 2 == 0:
                        nc.vector.tensor_copy(xp[:, :], ps[:, :])
                    else:
                        nc.scalar.activation(xp[:, :], ps[:, :], AF.Copy)
                    nc.sync.dma_start(out=cc["x2_in"][c8, et, :, :], in_=xp[:, :])
            nc.gpsimd.collective_compute(
                "ReduceScatter", mybir.AluOpType.add,
                replica_groups=[list(range(NC))],
                ins=[cc["x2_in"][:, :, :, :]], outs=[cc["x2_out"][:, :, :]])
            for et in range(ET):
                rsx = pd.tile([128, TOK], bf16, tag="rsx", name="rsx")
                nc.sync.dma_start(out=rsx[:, :], in_=cc["x2_out"][et, :, :])
                nc.vector.scalar_tensor_tensor(x2T[et][:, :], rsx[:, :],
                                               g["bo2_sb"][:, et:et + 1],
                                               x_sb[et][:, :], OP.add, OP.add)
            if dbg:
                for et in range(ET):
                    nc.sync.dma_start(out=dbg_out["x2T"][et, :, :], in_=x2T[et][:, :])

        with (
            tc.tile_pool(name="pe_sb", bufs=2) as pe,
            tc.tile_pool(name="pe_ps", bufs=1, space="PSUM") as peps,
        ):
            x2b = [pe.tile([128, TOK], bf16, tag=f"x2b{i}", name=f"x2b{i}")
                   for i in range(ET)]
            for et in range(ET):
                nc.vector.tensor_copy(x2b[et][:, :], x2T[et][:, :])
            layernorm_local(pe, peps, x2T, x2b, g["g2_sb"], g["b2_sb"], ln2_t, "e")
            for hi, half in enumerate(("A", "B")):
                hs = slice(256 * hi, 256 * (hi + 1))
                for et in range(ET):
                    nc.sync.dma_start(out=cc[f"ln2_in{half}"][et, :, :],
                                      in_=ln2_t[et][:, hs])
                nc.gpsimd.collective_compute(
                    "AllGather", mybir.AluOpType.bypass,
                    replica_groups=[list(range(NC))],
                    ins=[cc[f"ln2_in{half}"][:, :, :]],
                    outs=[cc[f"ln2_ag{half}"][:, :, :, :]])
            if dbg:
                for et in range(ET):
                    nc.sync.dma_start(out=dbg_out["ln2"][et, :, :], in_=ln2_t[et][:, :])

        # ---------------- Phase F: MLP hidden slice + ReduceScatter + residual ----------------
        with (
            tc.tile_pool(name="pf_mv", bufs=2) as mvpool,
            tc.tile_pool(name="pf_sb", bufs=4) as pf,
            tc.tile_pool(name="pf_ps", bufs=2, space="PSUM") as fps,
            tc.tile_pool(name="pf_ps2", bufs=4, space="PSUM") as pps,
        ):
            for hi, half in enumerate(("A", "B")):
                for c8 in range(NC):
                    mv8 = mvpool.tile([128, ET, TOK // 2], f8, tag="fmv8", name="fmv8")
                    nc.sync.dma_start(out=mv8[:, :, :],
                                      in_=cc[f"ln2_ag{half}"][c8].transpose([1, 0, 2]))
                    mv = mvpool.tile([128, ET, TOK // 2], bf16, tag="fmv", name="fmv")
                    nc.vector.tensor_copy(mv[:, :, :], mv8[:, :, :])
                    csl = slice(TOK * c8 + 256 * hi, TOK * c8 + 256 * (hi + 1))
                    for f in range(4):
                        ps = fps.tile([128, TOK // 2], f32, tag="fcps", name="fcps")
                        for et in range(ET):
                            nc.tensor.matmul(ps[:, :], g["wfc_sb"][:, et, f, :],
                                             mv[:, et, :],
                                             start=(et == 0), stop=(et == ET - 1))
                        nc.scalar.activation(h_sb[f][:, csl], ps[:, :], AF.Gelu,
                                             bias=g["bfc_sb"][:, f:f + 1])
            for et in range(ET):
                for c8 in range(NC):
                    ps = pps.tile([128, TOK], f32, tag="prps", name="prps")
                    for f in range(4):
                        nc.tensor.matmul(ps[:, :], g["wpr_sb"][:, f, et, :],
                                         h_sb[f][:, TOK * c8:TOK * (c8 + 1)],
                                         start=(f == 0), stop=(f == 3),
                                         skip_group_check=True)
                    yp = pf.tile([128, TOK], bf16, tag="yp", name="yp")
                    if c8 % 2 == 0:
                        nc.vector.tensor_copy(yp[:, :], ps[:, :])
                    else:
                        nc.scalar.activation(yp[:, :], ps[:, :], AF.Copy)
                    nc.sync.dma_start(out=cc["y_in"][c8, et, :, :], in_=yp[:, :])
            nc.gpsimd.collective_compute(
                "ReduceScatter", mybir.AluOpType.add,
                replica_groups=[list(range(NC))],
                ins=[cc["y_in"][:, :, :, :]], outs=[cc["y_out"][:, :, :]])
            for et in range(ET):
                yo = pf.tile([128, TOK], bf16, tag="yob", name="yob")
                nc.sync.dma_start(out=yo[:, :], in_=cc["y_out"][et, :, :])
                out_sb = pf.tile([128, TOK], f32, tag="osb", name="osb")
                nc.vector.scalar_tensor_tensor(out_sb[:, :], yo[:, :],
                                               g["bpr_sb"][:, et:et + 1],
                                               x2T[et][:, :], OP.add, OP.add)
                nc.sync.dma_start(out=g["yT"][et, :, :], in_=out_sb[:, :])


def _prep_inputs(x, ln1_g, ln1_b, ln2_g, ln2_b, W_qkv, b_qkv, W_o, b_o, W_fc, b_fc, W_proj, b_proj):
    f = np.float32
    bf = ml_dtypes.bfloat16
    x = np.asarray(x, f)
    W_qkv = np.asarray(W_qkv, f)
    b_qkv = np.asarray(b_qkv, f)
    W_o = np.asarray(W_o, f)
    W_fc = np.asarray(W_fc, f)
    W_proj = np.asarray(W_proj, f)
    scale = f(1.0) / f(np.sqrt(HEAD_DIM))
    Wq = W_qkv[:, :N_EMBD] * scale
    Wk = W_qkv[:, N_EMBD:2 * N_EMBD]
    Wv = W_qkv[:, 2 * N_EMBD:]
    bqv = b_qkv[:N_EMBD] * scale
    bkv = b_qkv[N_EMBD:2 * N_EMBD]
    bvv = b_qkv[2 * N_EMBD:]
    bo2 = np.asarray(b_o, f) + bvv @ W_o

    def ptile(vec, n):
        return np.ascontiguousarray(np.asarray(vec, f).reshape(n, 128).T)

    # sliding mask table M[i, c] = -1e9 iff i + 384 > c; diagonal-offset o
    # tile uses cols [384-128o : 384-128o+512]
    mask = np.zeros((128, 896), f)
    io = np.arange(128)[:, None] + 384
    jo = np.arange(896)[None, :]
    mask[io > jo] = f(-1e9)

    in_maps = []
    for core in range(NC):
        b, ch = divmod(core, 4)
        cs = slice(128 * core, 128 * (core + 1))
        m = dict(
            xT=np.ascontiguousarray(
                x[b, TOK * ch:TOK * (ch + 1), :].T.reshape(ET, 128, TOK)).astype(bf),
            wq=np.ascontiguousarray(
                Wq[:, cs].reshape(ET, 128, 128).transpose(1, 0, 2)).astype(bf),
            wk=np.ascontiguousarray(
                Wk[:, cs].reshape(ET, 128, 128).transpose(1, 0, 2)).astype(bf),
            wv=np.ascontiguousarray(
                Wv[:, cs].reshape(ET, 128, 128).transpose(1, 0, 2)).astype(bf),
            wo=np.ascontiguousarray(W_o[cs, :].reshape(128, ET, 128)).astype(bf),
            wfc=np.ascontiguousarray(
                W_fc[:, 512 * core:512 * (core + 1)]
                .reshape(ET, 128, 4, 128).transpose(1, 0, 2, 3)).astype(bf),
            wpr=np.ascontiguousarray(
                W_proj[512 * core:512 * (core + 1), :]
                .reshape(4, 128, ET, 128).transpose(1, 0, 2, 3)).astype(bf),
            bq=np.ascontiguousarray(bqv[cs].reshape(128, 1)),
            bk=np.ascontiguousarray(bkv[cs].reshape(128, 1)),
            bo2=ptile(bo2, ET),
            bfc=np.ascontiguousarray(
                np.asarray(b_fc, f)[512 * core:512 * (core + 1)].reshape(4, 128).T),
            bpr=ptile(b_proj, ET),
            g1=ptile(ln1_g, ET), b1=ptile(ln1_b, ET),
            g2=ptile(ln2_g, ET), b2=ptile(ln2_b, ET),
            dmask=mask,
        )
        in_maps.append(m)
    return in_maps


_CACHE = {}


def _get_built():
    if "nc" not in _CACHE:
        _CACHE["nc"] = build(stage=3, dbg=False, reps=1)[0]
    return _CACHE["nc"]


def kernel(**inputs):
    from concourse.bass_utils import run_bass_kernel_spmd

    nc = _get_built()
    in_maps = _prep_inputs(**inputs)
    res = run_bass_kernel_spmd(nc, in_maps, list(range(NC)))
    out = np.zeros((B, T, N_EMBD), np.float32)
    for core in range(NC):
        b, ch = divmod(core, 4)
        yt = res.results[core]["yT"].reshape(N_EMBD, TOK)
        out[b, TOK * ch:TOK * (ch + 1), :] = yt.T
    return out
